# revision 1
# baseline (speedup 1.0000x reference)
"""ChebConv (K=4) Trainium2 kernel.

Math (exactly matches the reference, which applies the spmm to `x` — not the
recurrence state — in every Chebyshev iteration):

    deg   = segment_sum(edge_weight, row)
    dinv  = deg^-1/2 (0 where deg <= 0)
    L[r,c]= sum over edges (r,c) of -2*dinv[r]*w*dinv[c];  L[i,i] += 2*fill
    Lx    = L @ x[b]                    (per batch)
    out   = x @ (W0 - W2) + Lx @ (W1 + 2*W2 + W3) + bias

Device strategy: densify L (pad N 10000->10240), shard its rows over 8 cores
(1280 rows each).  Per core the SpMM becomes a (1280 x 10240) @ (10240 x 512)
bf16 matmul (512 = 4 batches x 128 features concatenated) accumulated in fp32
PSUM, followed by fp32 feature-transform matmuls (x@A + Lx@B + bias) done in
transposed orientation so no device-side transpose of x is needed.
"""

import numpy as np
import ml_dtypes

B = 4
N_NODES = 10000
F = 128
SELF_LOOP_FILL = -0.05
NCORES = 8
NPAD = 10240                 # 80 tiles of 128; divisible by 8 cores
MROWS = NPAD // NCORES       # 1280 output rows per core
MT = MROWS // 128            # 10 m-tiles per core
KT = NPAD // 128             # 80 k-tiles
KCHUNK = 8                   # k-tiles per L/X DMA chunk
BF = B * F                   # 512 moving columns
# phase-1 m-groups: 6+4 m-tiles accumulate in parallel PSUM banks while k
# streams; group 0 is wide so its L-demand stays under the HBM rate while X
# streams alongside.
MGROUPS = [(0, 6), (6, 4)]           # (first m, count)
LM_OFFS = [0, KT * 6 * 128]          # free-dim offset of each group's L block
# k-chunk schedule: small leading chunks let the first matmuls start early
G0_CHUNKS = [2, 2, 4] + [8] * 9      # group 0 (also the X DMA schedule)
GN_CHUNKS = [8] * 10                 # later groups

_state = {}


def _build_nc():
    from contextlib import ExitStack

    import concourse.bass as bass
    import concourse.bacc as bacc
    import concourse.tile as tile
    from concourse import mybir

    dt = mybir.dt
    nc = bacc.Bacc(
        "TRN2", target_bir_lowering=False, debug=False, num_devices=NCORES
    )

    lmat = nc.declare_dram_parameter(
        "lmat", [128, KT * MROWS], dt.bfloat16, isOutput=False
    )
    xmat = nc.declare_dram_parameter("xmat", [128, KT, BF], dt.bfloat16, isOutput=False)
    xt = nc.declare_dram_parameter("xt", [128, B, MROWS], dt.float32, isOutput=False)
    wa = nc.declare_dram_parameter("wa", [128, 128], dt.float32, isOutput=False)
    wb = nc.declare_dram_parameter("wb", [128, 128], dt.float32, isOutput=False)
    biasv = nc.declare_dram_parameter("biasv", [128, 1], dt.float32, isOutput=False)
    ident = nc.declare_dram_parameter("ident", [128, 128], dt.float32, isOutput=False)
    out_t = nc.declare_dram_parameter("out_t", [B, 128, MROWS], dt.float32, isOutput=True)

    with ExitStack() as ctx:
        tc = ctx.enter_context(tile.TileContext(nc))
        const = ctx.enter_context(tc.tile_pool(name="const", bufs=1))
        xpool = ctx.enter_context(tc.tile_pool(name="xmat", bufs=1))
        lpool = ctx.enter_context(tc.tile_pool(name="lchunk", bufs=3))
        lxpool = ctx.enter_context(tc.tile_pool(name="lx", bufs=MT))
        lxtpool = ctx.enter_context(tc.tile_pool(name="lxt", bufs=1))
        outpool = ctx.enter_context(tc.tile_pool(name="outstg", bufs=3))
        # one shared PSUM pool: 6 phase-1 accumulators + warmup/transpose/
        # phase-2 tiles all rotate through the 8 banks
        psum = ctx.enter_context(
            tc.tile_pool(name="psum", bufs=8, space=bass.MemorySpace.PSUM)
        )
        psum1 = psumT = psum2 = psum

        # constants + xt on the scalar HWDGE queue (off the streaming path);
        # ident goes first — the PE warmup depends on it
        id_sb = const.tile([128, 128], dt.float32, tag="ident")
        nc.scalar.dma_start(id_sb[:], ident[:])
        wa_sb = const.tile([128, 128], dt.float32, tag="wa")
        nc.scalar.dma_start(wa_sb[:], wa[:])
        wb_sb = const.tile([128, 128], dt.float32, tag="wb")
        nc.scalar.dma_start(wb_sb[:], wb[:])
        bias_sb = const.tile([128, 1], dt.float32, tag="bias")
        nc.scalar.dma_start(bias_sb[:], biasv[:])
        xt_sb = const.tile([128, B, MROWS], dt.float32, tag="xt")

        xm_sb = xpool.tile([128, KT, BF], dt.bfloat16)
        lxT_sb = lxtpool.tile([128, B, MROWS], dt.float32)
        lx_tiles = [None] * MT

        # PE warmup: dummy matmuls on the identity as soon as it lands, so
        # the HAM clock-gate opens before the first real chunk arrives.
        pw = psum.tile([128, 128], dt.float32, tag="ps", name="ps_warm")
        for i in range(36):
            nc.tensor.matmul(
                pw[:], id_sb[:], id_sb[:], start=(i == 0), stop=(i == 35)
            )

        # Phase 1 (k-major): for each m-group, stream k-chunks of L (and, in
        # group 0, X) and accumulate the group's m-tiles in parallel PSUM
        # banks.
        for gi, (m0, gw) in enumerate(MGROUPS):
            wg = gw * 128
            chunks = G0_CHUNKS if gi == 0 else GN_CHUNKS
            pss = [
                psum.tile([128, BF], dt.float32, tag="ps", name=f"ps1_{m0 + i}")
                for i in range(gw)
            ]
            kt = 0
            off = LM_OFFS[gi]
            for kc in chunks:
                if gi == 0:
                    # X rides the scalar HWDGE queue so its prefetch is not
                    # gated by L's tile-slot flow control on the sync queue
                    nc.scalar.dma_start(
                        xm_sb[:, kt : kt + kc, :], xmat[:, kt : kt + kc, :]
                    )
                lt = lpool.tile([128, kc * wg], dt.bfloat16, tag="lt")
                nc.sync.dma_start(lt[:], lmat[:, off : off + kc * wg])
                off += kc * wg
                for j in range(kc):
                    for i in range(gw):
                        nc.tensor.matmul(
                            pss[i][:],
                            lt[:, j * wg + i * 128 : j * wg + (i + 1) * 128],
                            xm_sb[:, kt, :],
                            start=(kt == 0),
                            stop=(kt == KT - 1),
                        )
                    kt += 1
            for i in range(gw):
                lx_sb = lxpool.tile([128, BF], dt.float32, tag="lx")
                nc.vector.tensor_copy(lx_sb[:], pss[i][:])
                lx_tiles[m0 + i] = lx_sb

        # xt is only needed by phase 2 — load it while group 1 streams
        nc.scalar.dma_start(xt_sb[:], xt[:])

        # Phase 1.5: transpose Lx tiles (node-major -> feature-major)
        for m in range(MT):
            for b in range(B):
                pt = psumT.tile([128, 128], dt.float32, tag="ps", name=f"pt_{m}_{b}")
                nc.tensor.transpose(
                    pt[:], lx_tiles[m][:, b * 128 : (b + 1) * 128], id_sb[:]
                )
                nc.vector.tensor_copy(lxT_sb[:, b, m * 128 : (m + 1) * 128], pt[:])

        # Phase 2: out_T = A^T x^T + B^T Lx^T + bias  (fp32)
        starts = list(range(0, MROWS, 512))
        for b in range(B):
            for st in starts:
                csz = min(512, MROWS - st)
                ps2 = psum2.tile([128, 512], dt.float32, tag="ps", name=f"ps2_{b}_{st}")
                nc.tensor.matmul(
                    ps2[:, :csz], wa_sb[:], xt_sb[:, b, st : st + csz],
                    start=True, stop=False,
                )
                nc.tensor.matmul(
                    ps2[:, :csz], wb_sb[:], lxT_sb[:, b, st : st + csz],
                    start=False, stop=True,
                )
                ot = outpool.tile([128, 512], dt.float32, tag="ot")
                nc.scalar.activation(
                    ot[:, :csz], ps2[:, :csz],
                    mybir.ActivationFunctionType.Identity,
                    bias=bias_sb[:],
                )
                nc.scalar.dma_start(out_t[b, :, st : st + csz], ot[:, :csz])

    return nc


def _get_nc():
    if "nc" not in _state:
        nc = _build_nc()
        nc.compile()
        _state["nc"] = nc
    return _state["nc"]


def _prep_inputs(x, edge_index, edge_weight, weight, bias):
    """Host-side graph preprocessing -> per-core device input maps."""
    bf16 = ml_dtypes.bfloat16
    row = np.asarray(edge_index[0], dtype=np.int64)
    col = np.asarray(edge_index[1], dtype=np.int64)
    w = np.asarray(edge_weight, dtype=np.float32)

    deg = np.bincount(row, weights=w.astype(np.float64), minlength=N_NODES)
    deg = deg.astype(np.float32)
    dinv = np.where(deg > 0, np.where(deg > 0, deg, 1.0) ** -0.5, 0.0).astype(
        np.float32
    )
    lap2 = (-2.0 * dinv[row] * w * dinv[col]).astype(np.float32)

    # Dense transposed Laplacian: LT[src, dst] (lhsT orientation for the PE)
    LT = np.zeros((NPAD, NPAD), dtype=np.float32)
    np.add.at(LT, (col, row), lap2)
    idx = np.arange(N_NODES)
    LT[idx, idx] += 2.0 * SELF_LOOP_FILL
    LT16 = LT.astype(bf16)
    del LT

    # X in (node, batch*feat) layout, zero-padded rows
    xn = np.ascontiguousarray(np.transpose(x, (1, 0, 2)).reshape(N_NODES, BF))
    xn_pad = np.zeros((NPAD, BF), dtype=np.float32)
    xn_pad[:N_NODES] = xn
    # moving operand: (kr, kt, bf), node = kt*128 + kr
    xmat = np.ascontiguousarray(
        xn_pad.reshape(KT, 128, BF).transpose(1, 0, 2)
    ).astype(bf16)

    W = np.asarray(weight, dtype=np.float32)
    A = W[0] - W[2]
    Bm = W[1] + 2.0 * W[2] + W[3]
    biasv = np.asarray(bias, dtype=np.float32).reshape(128, 1)
    identity = np.eye(128, dtype=np.float32)

    in_maps = []
    for c in range(NCORES):
        r0, r1 = c * MROWS, (c + 1) * MROWS
        # per m-group block: [kr, kt, dst-in-group], groups concatenated on
        # the free dim; lmat[kr, off_g + (kt*gw*128 + dg)] = LT[kt*128+kr, ...]
        shard = LT16[:, r0:r1].reshape(KT, 128, MROWS)  # [kt, kr, dst]
        blocks = []
        for (m0, gw) in MGROUPS:
            blk = shard[:, :, m0 * 128 : (m0 + gw) * 128]  # (KT,128,gw*128)
            blocks.append(blk.transpose(1, 0, 2).reshape(128, KT * gw * 128))
        lmat = np.ascontiguousarray(np.concatenate(blocks, axis=1))
        # xt[f, b, nn] = x[b, r0+nn, f]
        xt = np.ascontiguousarray(xn_pad[r0:r1].reshape(MROWS, B, F).transpose(2, 1, 0))
        in_maps.append(
            {
                "lmat": lmat,
                "xmat": xmat,
                "xt": xt,
                "wa": A,
                "wb": Bm,
                "biasv": biasv,
                "ident": identity,
            }
        )
    return in_maps


def _ensure_ntff_hook():
    """Register the axon NTFF profiling hook if the image's antenv lacks it.

    The boot path degrades silently when ``antenv.axon_hooks`` is missing;
    recreate the tiny get/set holder and wire it to libaxon_pjrt.so so
    ``run_bass_kernel_spmd(trace=True)`` can capture NTFF profiles.
    """
    import sys
    import types

    try:
        from antenv.axon_hooks import get_axon_ntff_profile_hook  # noqa: F401

        return
    except ImportError:
        pass
    mod = types.ModuleType("antenv.axon_hooks")
    holder = {}
    mod.set_axon_ntff_profile_hook = lambda h: holder.__setitem__("h", h)
    mod.get_axon_ntff_profile_hook = lambda: holder.get("h")
    sys.modules["antenv.axon_hooks"] = mod
    import antenv

    antenv.axon_hooks = mod
    from trn_agent_boot.trn_boot import _ntff_profile_via_ctypes

    hook = _ntff_profile_via_ctypes("/opt/axon/libaxon_pjrt.so")
    if hook is not None:
        mod.set_axon_ntff_profile_hook(hook)


def kernel(x, edge_index, edge_weight, weight, bias):
    import os

    from concourse.bass_utils import run_bass_kernel_spmd

    x = np.asarray(x, dtype=np.float32)
    in_maps = _prep_inputs(x, edge_index, edge_weight, weight, bias)
    nc = _get_nc()
    trace = bool(int(os.environ.get("CHEB_TRACE", "0")))
    if trace:
        _ensure_ntff_hook()
    res = run_bass_kernel_spmd(nc, in_maps, list(range(NCORES)), trace=trace)
    _state["last_result"] = res
    out_T = np.concatenate([res.results[c]["out_t"] for c in range(NCORES)], axis=2)
    out = np.ascontiguousarray(out_T.transpose(0, 2, 1)[:, :N_NODES, :])
    return out

